# revision 43
# baseline (speedup 1.0000x reference)
"""Trainium2 Bass kernel for depthwise-spatial-conv:
out[b, i*D+d, 0, t] = sum_c maxnorm(w)[i*D+d, c] * x[b, i, c, t]

Sharding: data-parallel over batch (B=32 -> 4 per core across 8 cores),
weight replicated on every core.

Per core, each (b, i) is a tiny (8 x 128) @ (128 x 2048) matmul.
Structure: i-blocks are processed in groups of 4 via block-diagonal
(C x 32) weights, so each 4-matmul PSUM accumulation group yields a dense
(32, 512) tile at a 32-aligned partition base. The 4 groups sit in
distinct 32-col strips of the PE array (tile_position) and run
concurrently.

The matmul runs in bf16 (tolerance is 2e-2; bf16 lands ~1e-3): fp32
would stream the moving operand at 4 cycles/row and make PE the
bottleneck; bf16 streams at 1 cycle/row, leaving the kernel bound by
the HBM read of x (~358 GB/s/core). float32r was measured 1.85x WORSE
than fp32 (its fp32_mode=HIGH path gets no HAM warm-up credit and
streams at 4 cyc/row, and it forbids 32-col-strip concurrency).

Measured DMA facts (R=513 in-NEFF repeat-loop, all 8 cores concurrent):
  - x read stream (64 x 1 MiB linear HWDGE, bufs=8): ~190 us/iter
    = ~353 GB/s/core -- at the HBM-per-NC cap. Bigger tiles (2/4 MiB),
    a second HWDGE ring, and deeper bufs all measure the same.
  - pure writes run at ~370 GB/s (4 MiB in ~11.3 us), but interleaving
    the 4 batch stores into the saturated read stream costs ~7 us/iter
    in HBM bus-turnaround (dmarw 210.5 vs dmarwend 203.7 us). Hence
    store="mix" (default): batches 0-1 store mid-stream right after
    their drains (pays partial turnaround but hides 2 MiB of writes),
    batches 2-3 join an end burst that issues once the last load lands
    -- a 1-elem ACT op reading the last x tile holds the ACT FIFO
    until then, and the burst overlaps the SBUF-side compute tail.
    mix beat both all-interleaved (store=True, +2-4 us) and
    all-at-end (store="end", +1.5-2 us) consistently across processes;
    mix_end=1/3 measured no better than 2. Split-ring store bursts,
    512 KiB store pieces, gpsimd-ring stores, opening the gate a few
    loads early (gate_lead=2/4/7), sp_bufs=10/12, and a SWDGE
    cast-in-DMA rewrite of the whole pipeline (_body_swdge_mix: bf16
    lands straight from the x DMA, no staging/DVE-cast, bufs=16) all
    measured neutral or worse (+/-2 us noise floor; swdge +2 us from
    Q7 emission overhead).
  - half-T tiles for the last batch (shorter drain tail) measured
    ~5 us WORSE (32 extra half-size DMAs beat the tail saving); keep
    full-T tiles everywhere (last_ht=False).
  - compute is essentially free under the stream: casts +1 us, matmuls
    +0.4 us, drains +2 us (dmacast/nodrain/noout ladder). For_i loop
    overhead itself is ~0.5 us/iter (empty-body variant).

Cast modes:
  bf16_swdge - fp32->bf16 conversion inline in the x DMA (SWDGE/gpsimd
               path does dtype casts at stream rate).
  bf16_dve   - fp32 staged load (HWDGE) + DVE tensor_copy cast.
  fp32       - original exact-fp32 path (reference/baseline).
"""
import numpy as np

import concourse.bacc as bacc
import concourse.mybir as mybir
import concourse.tile as tile
from concourse.bass_utils import run_bass_kernel_spmd
from concourse.masks import make_identity

F32 = mybir.dt.float32
BF16 = mybir.dt.bfloat16

B, I, C, T, D = 32, 16, 128, 2048, 8
OUT_CH = I * D  # 128
N_CORES = 8
BPC = B // N_CORES  # batches per core
IG = 4            # i-blocks per DMA tile and per psum group
N_IG = I // IG    # 4
JT = 512          # matmul moving free-dim chunk (psum bank limit f32 out)
N_J = T // JT     # 4

MODE = "bf16_lin"

_CACHE = {}


def _preprocess_weights(nc, wp, pp, w):
    """DMA w, transpose to wT[c, oc] (unscaled), and compute the torch
    renorm(p=2, dim=0, maxnorm=1) scale as a per-out-channel (128,1)
    vector. The scale is applied during the PSUM-drain copies, so the
    sqrt/ACT-table chain stays off the first-matmul critical path."""
    w_sb = wp.tile([OUT_CH, C], F32)
    # ACT ring: keep the SP ring free so the first x load issues immediately
    nc.scalar.dma_start(out=w_sb[:, :], in_=w[:, 0, :, 0])
    sq = wp.tile([OUT_CH, C], F32)
    nc.vector.tensor_mul(sq[:, :], w_sb[:, :], w_sb[:, :])
    norm2 = wp.tile([OUT_CH, 1], F32)
    nc.vector.reduce_sum(out=norm2[:, :], in_=sq[:, :],
                         axis=mybir.AxisListType.X)
    norm = wp.tile([OUT_CH, 1], F32)
    nc.scalar.activation(out=norm[:, :], in_=norm2[:, :],
                         func=mybir.ActivationFunctionType.Sqrt,
                         bias=0.0, scale=1.0)
    nc.vector.tensor_scalar_max(norm[:, :], norm[:, :], 1e-12)
    inv = wp.tile([OUT_CH, 1], F32)
    nc.vector.reciprocal(inv[:, :], norm[:, :])
    nc.vector.tensor_scalar_min(inv[:, :], inv[:, :], 1.0)
    ident = wp.tile([128, 128], F32)
    make_identity(nc, ident[:, :])
    pt = pp.tile([128, 128], F32, tag="ps", bufs=8)
    nc.tensor.transpose(pt[:, :], w_sb[:, :], ident[:, :])
    return pt, inv


def _blockdiag4(nc, wp, wT, dtype, name):
    """t[:, i, :] is (C, 32): cols [8*(i%4), 8*(i%4)+8) = wT[:, 8i:8i+8),
    zero elsewhere (DVE casts fp32->dtype during the copies). A 4-matmul
    accumulation over i in one group fills a dense (32, JT) psum tile."""
    t = wp.tile([C, I, 32], dtype, name=name)
    nc.vector.memset(t[:, :, :], 0.0)
    for i in range(I):
        m = i % IG
        nc.vector.tensor_copy(t[:, i, m * D:(m + 1) * D],
                              wT[:, i * D:(i + 1) * D])
    return t


def _warmup_pe(nc, wp, pp):
    """HAM throttles a cold PE to 1.2 GHz until ~3.4us of sustained
    matmul activity. Burn that window during the initial DMA fill with
    dummy matmuls so the real stream starts at full clock."""
    wdum = wp.tile([128, 128], F32, name="wdum")
    nc.vector.memset(wdum[:, :], 0.5)
    psd = pp.tile([32, 128], F32, name="psd", tag="ps", bufs=8)
    for _ in range(12):
        nc.tensor.matmul(psd[:, :], wdum[:, :32], wdum[:, :],
                         start=True, stop=True)


def _mm_j_loop(nc, op, pp, xts, wbd4, scale, out_sb):
    """j-outer: each (b,j) fills one dense (128,512) psum bank; the 4
    i-groups land in distinct 32-col strips of the PE array
    (tile_position), so groups overlap in the array. One full-width
    scale-copy per (b,j) drains PSUM -> out_sb."""
    for j in range(N_J):
        sl = slice(j * JT, (j + 1) * JT)
        ps = pp.tile([128, JT], F32, name="psc", tag="ps", bufs=8)
        # half-chain strip interleave: strips switch every 2 matmuls so
        # consecutive instructions overlap in different col-strips
        order = [(g, m) for half in range(2)
                 for g in range(N_IG)
                 for m in (half * 2, half * 2 + 1)]
        for g, m in order:
            i = g * IG + m
            nc.tensor.matmul(
                ps[g * 32:(g + 1) * 32, :],
                wbd4[:, i, :], xts[g][:, m, sl],
                start=(m == 0), stop=(m == IG - 1),
                tile_position=(0, g * 32))
        nc.vector.tensor_scalar_mul(out_sb[:, sl], ps[:, :],
                                    scale[:, 0:1])


def _body_bf16_lin(nc, tc, x, w, o, sp_bufs=8, per_j_out=False, store="mix",
                   last_ht=False, end_gate=True, drain=True, gate_lead=0,
                   store_eng="scalar", mix_end=2, act_drain=True):
    """Linear-load + DVE-cast pipeline, bound by the x HBM read:

    - x is loaded per (b, i) as fully-linear 1 MiB (C, T) fp32 tiles on
      the SP HWDGE ring (measured ~356 GB/s/core with bufs>=8 -- deep
      queue of linear descriptors).
    - DVE casts each tile to bf16 right as it lands (~1.5 us/tile,
      hides under the ~3 us/tile DMA).
    - PE: g-outer sections -- each group of 4 bf16 tiles is consumed by
      its 16 matmuls (4 m x 4 psum banks) immediately, so tiles free
      after their own section and never gate the DMA stream on
      batch-tail compute. The 4 psum banks accumulate across sections
      in 32-row strips (start at g==0's m==0, stop at g==3's m==3 per
      strip); bank chains interleave, which per-element has_written
      tracking handles.
    """
    op_bufs = BPC if store == "end" else 3
    with tc.tile_pool(name="wp", bufs=1) as wp, \
         tc.tile_pool(name="sp", bufs=sp_bufs) as sp, \
         tc.tile_pool(name="xp", bufs=8) as xp, \
         tc.tile_pool(name="op", bufs=op_bufs) as op, \
         tc.tile_pool(name="pp", bufs=1, space="PSUM") as pp:
        wT, scale = _preprocess_weights(nc, wp, pp, w)
        wbd4 = _blockdiag4(nc, wp, wT, BF16, "wbd4")
        _warmup_pe(nc, wp, pp)

        out_sbs = []
        xs_seq = []
        for b in range(BPC):
            out_sb = op.tile([OUT_CH, T], F32, name="out_sb", tag="ob")
            out_sbs.append(out_sb)
            pss = [pp.tile([128, JT], F32, name=f"ps{j}", tag="ps", bufs=8)
                   for j in range(N_J)]
            if last_ht and b == BPC - 1:
                # Last batch in half-T tiles: the final drain chain hangs
                # off a (C, 1024) cast instead of (C, 2048), so the
                # end-store burst starts ~2-3us sooner after the last
                # load lands. 32 extra 512 KiB DMAs only for this batch.
                HT = T // 2
                for h in range(2):
                    for g in range(N_IG):
                        xbs = []
                        for m in range(IG):
                            i = g * IG + m
                            xs = sp.tile([C, HT], F32, name="xsh", tag="xs")
                            nc.sync.dma_start(
                                out=xs[:, :],
                                in_=x[b, i, :, h * HT:(h + 1) * HT])
                            xb = xp.tile([C, HT], BF16, name="xbh", tag="xb")
                            nc.vector.tensor_copy(xb[:, :], xs[:, :])
                            xbs.append(xb)
                        for m in range(IG):
                            i = g * IG + m
                            for jh in range(2):
                                j = 2 * h + jh
                                nc.tensor.matmul(
                                    pss[j][g * 32:(g + 1) * 32, :],
                                    wbd4[:, i, :],
                                    xbs[m][:, jh * JT:(jh + 1) * JT],
                                    start=(m == 0), stop=(m == IG - 1),
                                    tile_position=(0, g * 32),
                                    skip_group_check=True)
                    for jh in range(2):
                        j = 2 * h + jh
                        sl = slice(j * JT, (j + 1) * JT)
                        nc.vector.tensor_scalar_mul(out_sb[:, sl],
                                                    pss[j][:, :],
                                                    scale[:, 0:1])
                continue
            for g in range(N_IG):
                xbs = []
                for m in range(IG):
                    i = g * IG + m
                    xs = sp.tile([C, T], F32, name="xs", tag="xs")
                    nc.sync.dma_start(out=xs[:, :], in_=x[b, i])
                    xs_seq.append(xs)
                    xb = xp.tile([C, T], BF16, name="xb", tag="xb")
                    nc.vector.tensor_copy(xb[:, :], xs[:, :])
                    xbs.append(xb)
                for m in range(IG):
                    i = g * IG + m
                    for j in range(N_J):
                        nc.tensor.matmul(
                            pss[j][g * 32:(g + 1) * 32, :],
                            wbd4[:, i, :],
                            xbs[m][:, j * JT:(j + 1) * JT],
                            start=(m == 0), stop=(m == IG - 1),
                            tile_position=(0, g * 32),
                            skip_group_check=True)
            if not drain:
                continue
            if per_j_out == "last":
                do_j_out = (b == BPC - 1)
            else:
                do_j_out = per_j_out
            if do_j_out:
                # drain + store per j: the j-chunk out-DMA (256 KiB) issues
                # as soon as its drain lands, overlapping the remaining
                # j-chains' matmuls/drains -- shortens the exposed tail of
                # the last batch by ~3us vs one 1 MiB store at the end.
                for j in range(N_J):
                    sl = slice(j * JT, (j + 1) * JT)
                    nc.vector.tensor_scalar_mul(out_sb[:, sl], pss[j][:, :],
                                                scale[:, 0:1])
                    if store is True:
                        nc.scalar.dma_start(out=o[b, :, sl],
                                            in_=out_sb[:, sl])
            else:
                for j in range(N_J):
                    sl = slice(j * JT, (j + 1) * JT)
                    if act_drain and b < BPC - 1:
                        # mid-stream drains on the otherwise-idle ACT
                        # engine (activation Copy with per-partition
                        # scale, PSUM->SBUF): keeps DVE cast-only so
                        # casts never queue behind drains. Last batch
                        # stays on DVE so the tail drains run parallel
                        # to the ACT store burst.
                        nc.scalar.activation(
                            out=out_sb[:, sl], in_=pss[j][:, :],
                            func=mybir.ActivationFunctionType.Copy,
                            bias=0.0, scale=scale[:, 0:1])
                    else:
                        nc.vector.tensor_scalar_mul(out_sb[:, sl],
                                                    pss[j][:, :],
                                                    scale[:, 0:1])
                if store is True or (store == "mix" and b < BPC - mix_end):
                    # out-DMA on the ACT HWDGE ring: its sem wait (drain
                    # copies) must not stall the SP sequencer streaming the
                    # input loads
                    nc.scalar.dma_start(out=o[b, :, :], in_=out_sb[:, :])
        if store in ("end", "mix"):
            end_bs = (list(range(BPC)) if store == "end"
                      else list(range(BPC - mix_end, BPC)))
            # Segregate all stores after the read stream: interleaving
            # writes into the saturated read stream costs ~7us/iter in
            # HBM bus turnaround (dmarw 210.5us vs dmarwend 203.7us);
            # writes alone run at ~370 GB/s. A 1-elem ACT op reading the
            # last x tile holds the ACT FIFO until the final load lands;
            # the stores then issue in order 0..3, overlapping the
            # SBUF-side compute tail (casts/matmuls/drains) instead of
            # serializing after it. Batch 3's store is last, by which
            # time its drains are done.
            if end_gate:
                # gate_lead: open the store gate N loads before the end --
                # a small read/write overlap window beats both full
                # interleave and full segregation.
                gate_xs = xs_seq[len(xs_seq) - 1 - gate_lead]
                gate = wp.tile([1, 1], F32, name="gate")
                nc.scalar.activation(out=gate[:, :], in_=gate_xs[0:1, 0:1],
                                     func=mybir.ActivationFunctionType.Copy,
                                     bias=0.0, scale=1.0)
                order = end_bs
            else:
                # no gate: reverse order, so batch 3's drain RAW dep
                # holds the ACT FIFO until the whole compute tail ends
                order = reversed(end_bs)
            seng = getattr(nc, store_eng)
            for b in order:
                seng.dma_start(out=o[b, :, :], in_=out_sbs[b][:, :])


def _body_swdge_mix(nc, tc, x, w, o, xb_bufs=16, mix_end=2):
    """Like _body_bf16_lin with store="mix", but the fp32->bf16 cast
    happens inline in the x DMA (SWDGE/gpsimd ring): no fp32 staging
    pool, no DVE casts (DVE only drains), bf16 tiles double-buffered
    16 deep (64 KiB/partition -- twice the runway of the sp+xp split).
    HBM read traffic is unchanged (fp32 source)."""
    with tc.tile_pool(name="wp", bufs=1) as wp, \
         tc.tile_pool(name="xp", bufs=xb_bufs) as xp, \
         tc.tile_pool(name="op", bufs=BPC) as op, \
         tc.tile_pool(name="pp", bufs=1, space="PSUM") as pp:
        wT, scale = _preprocess_weights(nc, wp, pp, w)
        wbd4 = _blockdiag4(nc, wp, wT, BF16, "wbd4")
        _warmup_pe(nc, wp, pp)

        out_sbs = []
        last_xb = None
        for b in range(BPC):
            out_sb = op.tile([OUT_CH, T], F32, name="out_sb", tag="ob")
            out_sbs.append(out_sb)
            pss = [pp.tile([128, JT], F32, name=f"ps{j}", tag="ps", bufs=8)
                   for j in range(N_J)]
            for g in range(N_IG):
                xbs = []
                for m in range(IG):
                    i = g * IG + m
                    xb = xp.tile([C, T], BF16, name="xb", tag="xb")
                    nc.gpsimd.dma_start(out=xb[:, :], in_=x[b, i])
                    last_xb = xb
                    xbs.append(xb)
                for m in range(IG):
                    i = g * IG + m
                    for j in range(N_J):
                        nc.tensor.matmul(
                            pss[j][g * 32:(g + 1) * 32, :],
                            wbd4[:, i, :],
                            xbs[m][:, j * JT:(j + 1) * JT],
                            start=(m == 0), stop=(m == IG - 1),
                            tile_position=(0, g * 32),
                            skip_group_check=True)
            for j in range(N_J):
                sl = slice(j * JT, (j + 1) * JT)
                nc.vector.tensor_scalar_mul(out_sb[:, sl], pss[j][:, :],
                                            scale[:, 0:1])
            if b < BPC - mix_end:
                nc.scalar.dma_start(out=o[b, :, :], in_=out_sb[:, :])
        gate = wp.tile([1, 1], F32, name="gate")
        nc.scalar.activation(out=gate[:, :], in_=last_xb[0:1, 0:1],
                             func=mybir.ActivationFunctionType.Copy,
                             bias=0.0, scale=1.0)
        for b in range(BPC - mix_end, BPC):
            nc.scalar.dma_start(out=o[b, :, :], in_=out_sbs[b][:, :])


def _body(nc, tc, x, w, o, mode=None):
    mode = mode or MODE
    if mode == "bf16_lin":
        return _body_bf16_lin(nc, tc, x, w, o)
    if mode == "swdge_mix":
        return _body_swdge_mix(nc, tc, x, w, o)
    mm_dt = F32 if mode == "fp32" else BF16
    xt_bufs = {"fp32": 5, "bf16_swdge": 8, "bf16_dve": 5}[mode]
    with tc.tile_pool(name="wp", bufs=1) as wp, \
         tc.tile_pool(name="xp", bufs=xt_bufs) as xp, \
         tc.tile_pool(name="sp", bufs=2) as sp, \
         tc.tile_pool(name="op", bufs=3) as op, \
         tc.tile_pool(name="pp", bufs=1, space="PSUM") as pp:
        wT, scale = _preprocess_weights(nc, wp, pp, w)
        wbd4 = _blockdiag4(nc, wp, wT, mm_dt, "wbd4")
        _warmup_pe(nc, wp, pp)

        for b in range(BPC):
            out_sb = op.tile([OUT_CH, T], F32, name="out_sb", tag="ob")
            xts = []
            for g in range(N_IG):
                xt = xp.tile([C, IG, T], mm_dt, name=f"xt{g}", tag="xt")
                src = x[b, g * IG:(g + 1) * IG].rearrange("i c t -> c i t")
                if mode == "bf16_swdge":
                    # cast-in-DMA: SWDGE converts fp32->bf16 at stream
                    # rate; HBM read traffic is unchanged (fp32 source)
                    if b == 0:
                        for m in range(IG):
                            nc.gpsimd.dma_start(
                                out=xt[:, m, :],
                                in_=x[b, g * IG + m].rearrange("c t -> c t"))
                    else:
                        nc.gpsimd.dma_start(out=xt[:, :, :], in_=src)
                elif mode == "bf16_dve":
                    xs = sp.tile([C, IG, T], F32, name=f"xs{g}", tag="xs")
                    if b == 0:
                        for m in range(IG):
                            nc.sync.dma_start(
                                out=xs[:, m, :],
                                in_=x[b, g * IG + m].rearrange("c t -> c t"))
                    else:
                        nc.sync.dma_start(out=xs[:, :, :], in_=src)
                    for m in range(IG):
                        nc.vector.tensor_copy(xt[:, m, :], xs[:, m, :])
                else:
                    if b == 0 and g == 0:
                        # piecewise first tile: the first matmul chain
                        # starts after 1 MiB instead of 4 MiB
                        for m in range(IG):
                            nc.sync.dma_start(
                                out=xt[:, m, :],
                                in_=x[b, g * IG + m].rearrange("c t -> c t"))
                    else:
                        nc.sync.dma_start(out=xt[:, :, :], in_=src)
                xts.append(xt)
            _mm_j_loop(nc, op, pp, xts, wbd4, scale, out_sb)
            # out-DMA on the ACT HWDGE ring: its sem wait (drain copies)
            # must not stall the SP sequencer streaming the input loads
            nc.scalar.dma_start(out=o[b, :, :], in_=out_sb[:, :])


def _build(mode=None):
    nc = bacc.Bacc()
    x = nc.declare_dram_parameter("x", [BPC, I, C, T], F32, isOutput=False)
    w = nc.declare_dram_parameter("w", [OUT_CH, 1, C, 1], F32, isOutput=False)
    o = nc.declare_dram_parameter("o", [BPC, OUT_CH, T], F32, isOutput=True)

    with tile.TileContext(nc) as tc:
        _body(nc, tc, x, w, o, mode=mode)

    if not nc.is_finalized():
        nc.finalize()
    return nc


def _get_nc():
    if "nc" not in _CACHE:
        _CACHE["nc"] = _build()
    return _CACHE["nc"]


def _run(x, weight, **kw):
    assert x.shape == (B, I, C, T) and x.dtype == np.float32
    assert weight.shape == (OUT_CH, 1, C, 1) and weight.dtype == np.float32
    nc = _get_nc()
    in_maps = [
        {"x": np.ascontiguousarray(x[k * BPC:(k + 1) * BPC]), "w": weight}
        for k in range(N_CORES)
    ]
    res = run_bass_kernel_spmd(nc, in_maps, list(range(N_CORES)), **kw)
    out = np.concatenate([res.results[k]["o"] for k in range(N_CORES)], axis=0)
    return out.reshape(B, OUT_CH, 1, T), res


def kernel(x, weight):
    out, _ = _run(x, weight)
    return out

